# revision 8
# baseline (speedup 1.0000x reference)
"""Trainium2 Bass kernel for nn_CNN_80221399155117.

Pipeline: full-vocab softmax -> token-prob gather -> -log2 surprisal ->
concat(hidden, surp) -> Conv1d(k=5, pad=2) -> MaxPool1d(5) -> ReLU -> FC.

Sharding: 8 cores = (batch b, seq-half h). Each core owns the pool-aligned
conv-output range [510h, 510h+510) of its batch, needing feats rows
[510h-2, 510h+512) (EXT=514, zero-padded outside [0,1024)). The softmax
normalizer is computed locally per row (positions sharded, vocab local),
so no collectives are needed. The token-logit gather runs on-device via
indirect DMA with flat indices built from iota + input_ids.
"""

import numpy as np

B, S, V, H = 4, 1024, 32000, 2048
OC, K = 128, 5
N_CORES = 8
Y_LOC = 510            # conv output positions per core (102 pool windows)
PO_LOC = 102           # pooled cols per core
EXT = 514              # feats rows incl conv halo (510 + 2 + 2)
CF = 4000              # vocab chunk (free-dim) size
NCH = V // CF          # 8 chunks
LOG2E = 1.4426950408889634

_CACHE = {}
VARIANT = "indirect"   # bisect knob: indirect | nogather | flat2d | nopass1 | noconv


def _build_program():
    import concourse.tile as tile
    from concourse import bacc, bass, mybir
    from concourse.masks import make_identity

    f32 = mybir.dt.float32
    i32 = mybir.dt.int32
    Alu = mybir.AluOpType
    Act = mybir.ActivationFunctionType

    nc = bacc.Bacc("TRN2", target_bir_lowering=False, debug=False,
                   num_devices=N_CORES)

    logits = nc.dram_tensor("logits_loc", [EXT, V], f32, kind="ExternalInput").ap()
    ids = nc.dram_tensor("ids_loc", [EXT, 1], i32, kind="ExternalInput").ap()
    maskd = nc.dram_tensor("mask_loc", [EXT, 1], f32, kind="ExternalInput").ap()
    hid = nc.dram_tensor("hidden_loc", [EXT, H], f32, kind="ExternalInput").ap()
    wt = nc.dram_tensor("wt", [H, K * OC], f32, kind="ExternalInput").ap()
    wsurp = nc.dram_tensor("wsurp", [K, OC], f32, kind="ExternalInput").ap()
    convb = nc.dram_tensor("convb", [OC, 1], f32, kind="ExternalInput").ap()
    fcw = nc.dram_tensor("fcw", [OC, 3 * PO_LOC], f32, kind="ExternalInput").ap()
    sentv = nc.dram_tensor("sentv", [128, 1], f32, kind="ExternalInput").ap()
    sentw = nc.dram_tensor("sentw", [128, 3], f32, kind="ExternalInput").ap()
    fcb = nc.dram_tensor("fcb", [3, 1], f32, kind="ExternalInput").ap()
    out = nc.dram_tensor("out_loc", [3, 1], f32, kind="ExternalOutput").ap()

    surp_dram = nc.dram_tensor("surp_scratch", [1, EXT], f32).ap()

    logits_flat = bass.AP(logits.tensor, 0, [[1, EXT * V], [1, 1]])

    ROW_TILES = [(0, 128), (128, 128), (256, 128), (384, 128), (512, EXT - 512)]

    with tile.TileContext(nc) as tc:
        with (
            tc.tile_pool(name="lp", bufs=3) as lp,          # logits chunks
            tc.tile_pool(name="scr", bufs=2) as scr,        # exp scratch
            tc.tile_pool(name="big", bufs=1) as big,        # resident X / weights
            tc.tile_pool(name="hn", bufs=2) as hnp,         # hidden natural tiles
            tc.tile_pool(name="sm", bufs=12) as sm,         # small per-tile stats
            tc.tile_pool(name="ps_t", bufs=4, space="PSUM") as ps_t,
            tc.tile_pool(name="ps_y", bufs=1, space="PSUM") as ps_y,
            tc.tile_pool(name="ps_o", bufs=1, space="PSUM") as ps_o,
        ):
            # ---- resident constants ----
            ident = big.tile([128, 128], f32, tag="ident")
            make_identity(nc, ident[:])
            wtile = big.tile([128, 16 * K * OC], f32, tag="wtile")  # 16 ch-chunks
            for cc in range(16):
                nc.sync.dma_start(
                    out=wtile[:, cc * 640:(cc + 1) * 640],
                    in_=wt[cc * 128:(cc + 1) * 128, :],
                )
            wsurp_sb = big.tile([K, OC], f32, tag="wsurp")
            nc.sync.dma_start(out=wsurp_sb[:], in_=wsurp)
            convb_sb = big.tile([OC, 1], f32, tag="convb")
            nc.sync.dma_start(out=convb_sb[:], in_=convb)
            fcw_sb = big.tile([OC, 3 * PO_LOC], f32, tag="fcw")
            nc.sync.dma_start(out=fcw_sb[:], in_=fcw)
            sentv_sb = big.tile([128, 1], f32, tag="sentv")
            nc.sync.dma_start(out=sentv_sb[:], in_=sentv)
            sentw_sb = big.tile([128, 3], f32, tag="sentw")
            nc.sync.dma_start(out=sentw_sb[:], in_=sentw)
            fcb_sb = big.tile([3, 1], f32, tag="fcb")
            nc.sync.dma_start(out=fcb_sb[:], in_=fcb)
            ones_sb = big.tile([128, 1], f32, tag="ones")
            nc.vector.memset(ones_sb[:], 1.0)

            # ---- hidden -> transposed X tiles [ch, pos] ----
            xt = big.tile([128, 16 * EXT], f32, tag="xt")
            for r0, pn in ROW_TILES:
                hn = hnp.tile([128, H], f32, tag="hn")
                nc.sync.dma_start(out=hn[:pn, :], in_=hid[r0:r0 + pn, :])
                for cc in range(16):
                    tp = ps_t.tile([128, 128], f32, tag="tp")
                    nc.tensor.transpose(
                        out=tp[:, :pn],
                        in_=hn[:pn, cc * 128:(cc + 1) * 128],
                        identity=ident[:pn, :pn],
                    )
                    nc.vector.tensor_copy(
                        out=xt[:, cc * EXT + r0: cc * EXT + r0 + pn],
                        in_=tp[:, :pn],
                    )

            # ---- conv: 80 hidden matmuls accumulate into one PSUM bank ----
            psum_y = ps_y.tile([OC, Y_LOC], f32, tag="y")
            first = True
            for cc in range(16):
                for k in range(K):
                    nc.tensor.matmul(
                        out=psum_y[:],
                        lhsT=wtile[:, cc * 640 + k * 128: cc * 640 + (k + 1) * 128],
                        rhs=xt[:, cc * EXT + k: cc * EXT + k + Y_LOC],
                        start=first,
                        stop=False,
                    )
                    first = False

            # ---- pass 1: surprisal for EXT rows ----
            for r0, pn in ROW_TILES:
                ids_sb = sm.tile([128, 1], i32, tag="ids")
                nc.sync.dma_start(out=ids_sb[:pn, :], in_=ids[r0:r0 + pn, :])
                mask_sb = sm.tile([128, 1], f32, tag="mask")
                nc.sync.dma_start(out=mask_sb[:pn, :], in_=maskd[r0:r0 + pn, :])

                rowbase = sm.tile([128, 1], i32, tag="rowbase")
                nc.gpsimd.iota(rowbase[:pn, :], pattern=[[1, 1]],
                               base=r0 * V, channel_multiplier=V)
                flat_ids = sm.tile([128, 1], i32, tag="flatids")
                nc.vector.tensor_tensor(out=flat_ids[:pn, :], in0=ids_sb[:pn, :],
                                        in1=rowbase[:pn, :], op=Alu.add)
                gath = sm.tile([128, 1], f32, tag="gath")
                if VARIANT == "nogather":
                    nc.vector.memset(gath[:pn, :], 0.0)
                elif VARIANT == "flat2d":
                    nc.gpsimd.indirect_dma_start(
                        out=gath[:pn, :],
                        out_offset=None,
                        in_=logits,
                        in_offset=bass.IndirectOffsetOnAxis(
                            ap=flat_ids[:pn, :1], axis=1),
                    )
                else:
                    nc.gpsimd.indirect_dma_start(
                        out=gath[:pn, :],
                        out_offset=None,
                        in_=logits_flat,
                        in_offset=bass.IndirectOffsetOnAxis(
                            ap=flat_ids[:pn, :1], axis=0),
                    )

                sums = sm.tile([128, NCH], f32, tag="sums")
                for ci in range(NCH):
                    x_sb = lp.tile([128, CF], f32, tag="x")
                    nc.sync.dma_start(
                        out=x_sb[:pn, :],
                        in_=logits[r0:r0 + pn, ci * CF:(ci + 1) * CF],
                    )
                    e_sb = scr.tile([128, CF], f32, tag="e")
                    nc.scalar.activation(
                        out=e_sb[:pn, :], in_=x_sb[:pn, :], func=Act.Exp,
                        accum_out=sums[:pn, ci:ci + 1],
                    )

                sumexp = sm.tile([128, 1], f32, tag="sumexp")
                nc.vector.tensor_reduce(
                    out=sumexp[:pn, :], in_=sums[:pn, :],
                    axis=mybir.AxisListType.X, op=Alu.add,
                )
                lse = sm.tile([128, 1], f32, tag="lse")
                nc.scalar.activation(out=lse[:pn, :], in_=sumexp[:pn, :], func=Act.Ln)

                surp = sm.tile([128, 1], f32, tag="surp")
                nc.vector.tensor_tensor(out=surp[:pn, :], in0=lse[:pn, :],
                                        in1=gath[:pn, :], op=Alu.subtract)
                nc.vector.tensor_tensor(out=surp[:pn, :], in0=surp[:pn, :],
                                        in1=mask_sb[:pn, :], op=Alu.mult)
                nc.vector.tensor_scalar(out=surp[:pn, :], in0=surp[:pn, :],
                                        scalar1=LOG2E, scalar2=None, op0=Alu.mult)
                nc.sync.dma_start(out=surp_dram[0:1, r0:r0 + pn], in_=surp[:pn, :])

            # ---- surp channel: 5 rank-1-ish matmuls close the accumulation ----
            s5 = big.tile([K, Y_LOC], f32, tag="s5")
            for k in range(K):
                nc.sync.dma_start(out=s5[k:k + 1, :],
                                  in_=surp_dram[0:1, k:k + Y_LOC])
            nc.tensor.matmul(
                out=psum_y[:],
                lhsT=wsurp_sb[:],
                rhs=s5[:],
                start=False,
                stop=True,
            )

            # ---- maxpool(5) + bias + relu ----
            pooled = big.tile([OC, PO_LOC], f32, tag="pooled")
            stop_off = K * (PO_LOC - 1) + 1
            nc.vector.tensor_copy(out=pooled[:], in_=psum_y[:, 0:stop_off:K])
            for j in range(1, K):
                nc.vector.tensor_tensor(out=pooled[:], in0=pooled[:],
                                        in1=psum_y[:, j:j + stop_off:K], op=Alu.max)
            nc.vector.tensor_scalar(out=pooled[:], in0=pooled[:],
                                    scalar1=convb_sb[:, 0:1], scalar2=None,
                                    op0=Alu.add)
            nc.vector.tensor_scalar(out=pooled[:], in0=pooled[:],
                                    scalar1=0.0, scalar2=None, op0=Alu.max)

            # ---- FC partial: red[oc, l] = sum_p pooled*fcw ----
            red = big.tile([OC, 3], f32, tag="red")
            fc_scr = big.tile([OC, PO_LOC], f32, tag="fcscr")
            for l in range(3):
                nc.vector.tensor_tensor(
                    out=fc_scr[:],
                    in0=pooled[:],
                    in1=fcw_sb[:, l * PO_LOC:(l + 1) * PO_LOC],
                    op=Alu.mult,
                )
                nc.vector.tensor_reduce(
                    out=red[:, l:l + 1], in_=fc_scr[:],
                    axis=mybir.AxisListType.X, op=Alu.add,
                )
            # sentiment branch (zeroed on h==1 cores)
            rs = sm.tile([128, 1], f32, tag="rs")
            nc.vector.tensor_scalar(out=rs[:], in0=sentv_sb[:], scalar1=0.0,
                                    scalar2=None, op0=Alu.max)
            tmp3 = sm.tile([128, 3], f32, tag="tmp3")
            nc.vector.tensor_scalar(out=tmp3[:], in0=sentw_sb[:],
                                    scalar1=rs[:, 0:1], scalar2=None, op0=Alu.mult)
            nc.vector.tensor_tensor(out=red[:], in0=red[:], in1=tmp3[:], op=Alu.add)

            psum_out = ps_o.tile([3, 1], f32, tag="po")
            nc.tensor.matmul(out=psum_out[:], lhsT=red[:], rhs=ones_sb[:],
                             start=True, stop=True)
            out_sb = sm.tile([3, 1], f32, tag="outsb")
            nc.vector.tensor_tensor(out=out_sb[:], in0=psum_out[:], in1=fcb_sb[:],
                                    op=Alu.add)
            nc.sync.dma_start(out=out, in_=out_sb[:])

    nc.compile()
    return nc


def _prep_core_inputs(core, input_ids, attention_mask, sentiment, logits,
                      hidden, conv_w, conv_b, fc_w, fc_b):
    b, h = core // 2, core % 2
    g0 = Y_LOC * h
    ext0 = g0 - 2

    lg = np.zeros((EXT, V), np.float32)
    idl = np.zeros((EXT, 1), np.int32)
    mk = np.zeros((EXT, 1), np.float32)
    hd = np.zeros((EXT, H), np.float32)
    lo = max(0, -ext0)            # local index where valid rows start
    s0, s1 = ext0 + lo, ext0 + EXT
    lg[lo:] = logits[b, s0:s1]
    idl[lo:, 0] = input_ids[b, s0:s1].astype(np.int32)
    mk[lo:, 0] = attention_mask[b, s0:s1]
    hd[lo:] = hidden[b, s0:s1]

    wt = np.ascontiguousarray(
        conv_w[:, :H, :].transpose(1, 2, 0).reshape(H, K * OC))
    ws = np.ascontiguousarray(conv_w[:, H, :].T)           # [K, OC]
    cb = np.ascontiguousarray(conv_b[:, None])             # [OC, 1]

    w3 = fc_w[:, :OC * 204].reshape(3, OC, 204)
    fcw = np.ascontiguousarray(
        w3[:, :, h * PO_LOC:(h + 1) * PO_LOC].transpose(1, 0, 2).reshape(OC, 3 * PO_LOC))

    sv = np.zeros((128, 1), np.float32)
    sw = np.zeros((128, 3), np.float32)
    fb = np.zeros((3, 1), np.float32)
    if h == 0:
        sv[:3, 0] = sentiment[b]
        sw[:3, :] = fc_w[:, OC * 204:].T                   # [3 j, 3 l]
        fb[:, 0] = fc_b

    return {
        "logits_loc": lg, "ids_loc": idl, "mask_loc": mk, "hidden_loc": hd,
        "wt": wt, "wsurp": ws, "convb": cb, "fcw": fcw,
        "sentv": sv, "sentw": sw, "fcb": fb,
    }


def _install_ntff_hook():
    import sys
    import types
    try:
        import antenv
        from trn_agent_boot.trn_boot import _ntff_profile_via_ctypes
    except ImportError:
        return
    if "antenv.axon_hooks" in sys.modules:
        return
    mod = types.ModuleType("antenv.axon_hooks")
    _h = [None]
    mod.set_axon_ntff_profile_hook = lambda hk: _h.__setitem__(0, hk)
    mod.get_axon_ntff_profile_hook = lambda: _h[0]
    sys.modules["antenv.axon_hooks"] = mod
    antenv.axon_hooks = mod
    try:
        mod.set_axon_ntff_profile_hook(
            _ntff_profile_via_ctypes('/opt/axon/libaxon_pjrt.so'))
    except Exception:
        pass


def kernel(input_ids, attention_mask, sentiment, logits, hidden,
           conv_w, conv_b, fc_w, fc_b, _trace=False):
    from concourse.bass_utils import run_bass_kernel_spmd

    input_ids = np.asarray(input_ids)
    attention_mask = np.asarray(attention_mask, np.float32)
    sentiment = np.asarray(sentiment, np.float32)
    logits = np.asarray(logits, np.float32)
    hidden = np.asarray(hidden, np.float32)
    conv_w = np.asarray(conv_w, np.float32)
    conv_b = np.asarray(conv_b, np.float32)
    fc_w = np.asarray(fc_w, np.float32)
    fc_b = np.asarray(fc_b, np.float32)

    if "nc" not in _CACHE:
        _CACHE["nc"] = _build_program()
    nc = _CACHE["nc"]

    in_maps = [
        _prep_core_inputs(c, input_ids, attention_mask, sentiment, logits,
                          hidden, conv_w, conv_b, fc_w, fc_b)
        for c in range(N_CORES)
    ]
    if _trace:
        _install_ntff_hook()
    res = run_bass_kernel_spmd(nc, in_maps, list(range(N_CORES)), trace=_trace)
    _CACHE["last_result"] = res

    out = np.zeros((B, 3), np.float32)
    for b in range(B):
        out[b] = (res.results[2 * b]["out_loc"][:, 0]
                  + res.results[2 * b + 1]["out_loc"][:, 0])
    return out


# revision 12
# speedup vs baseline: 1.1256x; 1.1256x over previous
"""Trainium2 Bass kernel for nn_CNN_80221399155117.

Pipeline: full-vocab softmax -> token-prob gather -> -log2 surprisal ->
concat(hidden, surp) -> Conv1d(k=5, pad=2) -> MaxPool1d(5) -> ReLU -> FC.

Sharding: 8 cores = (batch b, seq-half h). Each core owns the pool-aligned
conv-output range [510h, 510h+510) of its batch, needing feats rows
[510h-2, 510h+512) (EXT=514, zero-padded outside [0,1024)). The softmax
normalizer is computed locally per row (positions sharded, vocab local),
so no collectives are needed. The token-logit gather runs on-device via
indirect DMA with flat indices built from iota + input_ids.
"""

import numpy as np

B, S, V, H = 4, 1024, 32000, 2048
OC, K = 128, 5
N_CORES = 8
Y_LOC = 510            # conv output positions per core (102 pool windows)
PO_LOC = 102           # pooled cols per core
EXT = 514              # feats rows incl conv halo (510 + 2 + 2)
CF = 4000              # vocab chunk (free-dim) size
NCH = V // CF          # 8 chunks
LOG2E = 1.4426950408889634

_CACHE = {}
VARIANT = "indirect"   # bisect knob: indirect | nogather | flat2d | nopass1 | noconv


def _build_program():
    import concourse.tile as tile
    from concourse import bacc, bass, mybir
    from concourse.masks import make_identity

    f32 = mybir.dt.float32
    i32 = mybir.dt.int32
    Alu = mybir.AluOpType
    Act = mybir.ActivationFunctionType

    nc = bacc.Bacc("TRN2", target_bir_lowering=False, debug=False,
                   num_devices=N_CORES)

    logits = nc.dram_tensor("logits_loc", [EXT, V], f32, kind="ExternalInput").ap()
    ids = nc.dram_tensor("ids_loc", [EXT, 1], i32, kind="ExternalInput").ap()
    maskd = nc.dram_tensor("mask_loc", [EXT, 1], f32, kind="ExternalInput").ap()
    hid = nc.dram_tensor("hidden_loc", [EXT, H], f32, kind="ExternalInput").ap()
    wt = nc.dram_tensor("wt", [H, K * OC], f32, kind="ExternalInput").ap()
    wsurp = nc.dram_tensor("wsurp", [K, OC], f32, kind="ExternalInput").ap()
    convb = nc.dram_tensor("convb", [OC, 1], f32, kind="ExternalInput").ap()
    fcw = nc.dram_tensor("fcw", [OC, 3 * PO_LOC], f32, kind="ExternalInput").ap()
    sentv = nc.dram_tensor("sentv", [128, 1], f32, kind="ExternalInput").ap()
    sentw = nc.dram_tensor("sentw", [128, 3], f32, kind="ExternalInput").ap()
    fcb = nc.dram_tensor("fcb", [3, 1], f32, kind="ExternalInput").ap()
    out = nc.dram_tensor("out_loc", [3, 1], f32, kind="ExternalOutput").ap()

    surp_dram = nc.dram_tensor("surp_scratch", [1, EXT], f32).ap()

    logits_flat = bass.AP(logits.tensor, 0, [[1, EXT * V], [1, 1]])

    # halo mini-tile first: its Exp instrs are free-dim-bound (cost like a
    # full tile) — run them under the main stream instead of as a tail
    ROW_TILES = [(512, EXT - 512), (0, 128), (128, 128), (256, 128), (384, 128)]

    with tile.TileContext(nc) as tc:
        with (
            tc.tile_pool(name="lp", bufs=5) as lp,          # logits chunks
            tc.tile_pool(name="scr", bufs=2) as scr,        # exp scratch
            tc.tile_pool(name="big", bufs=1) as big,        # resident X / weights
            tc.tile_pool(name="hn", bufs=2) as hnp,         # hidden natural tiles
            tc.tile_pool(name="sm", bufs=12) as sm,         # small per-tile stats
            tc.tile_pool(name="ps_t", bufs=4, space="PSUM") as ps_t,
            tc.tile_pool(name="ps_y", bufs=1, space="PSUM") as ps_y,
            tc.tile_pool(name="ps_o", bufs=1, space="PSUM") as ps_o,
        ):
            # ---- resident constants ----
            ident = big.tile([128, 128], f32, tag="ident")
            make_identity(nc, ident[:])
            bf16 = mybir.dt.bfloat16
            wtile = big.tile([128, 16 * K * OC], bf16, tag="wtile")  # 16 ch-chunks
            for cc in range(16):
                nc.gpsimd.dma_start(        # SWDGE casts f32->bf16 in flight
                    out=wtile[:, cc * 640:(cc + 1) * 640],
                    in_=wt[cc * 128:(cc + 1) * 128, :],
                )
            wsurp_sb = big.tile([K, OC], f32, tag="wsurp")
            nc.sync.dma_start(out=wsurp_sb[:], in_=wsurp)
            convb_sb = big.tile([OC, 1], f32, tag="convb")
            nc.sync.dma_start(out=convb_sb[:], in_=convb)
            fcw_sb = big.tile([OC, 3 * PO_LOC], f32, tag="fcw")
            nc.sync.dma_start(out=fcw_sb[:], in_=fcw)
            sentv_sb = big.tile([128, 1], f32, tag="sentv")
            nc.sync.dma_start(out=sentv_sb[:], in_=sentv)
            sentw_sb = big.tile([128, 3], f32, tag="sentw")
            nc.sync.dma_start(out=sentw_sb[:], in_=sentw)
            fcb_sb = big.tile([3, 1], f32, tag="fcb")
            nc.sync.dma_start(out=fcb_sb[:], in_=fcb)
            ones_sb = big.tile([128, 1], f32, tag="ones")
            nc.vector.memset(ones_sb[:], 1.0)

            # ---- hidden -> transposed X tiles [ch, pos] ----
            xt = big.tile([128, 16 * EXT], bf16, tag="xt")
            for r0, pn in ROW_TILES:
                hn = hnp.tile([128, H], f32, tag="hn")
                nc.sync.dma_start(out=hn[:pn, :], in_=hid[r0:r0 + pn, :])
                for cc in range(16):
                    tp = ps_t.tile([128, 128], f32, tag="tp")
                    nc.tensor.transpose(
                        out=tp[:, :pn],
                        in_=hn[:pn, cc * 128:(cc + 1) * 128],
                        identity=ident[:pn, :pn],
                    )
                    nc.vector.tensor_copy(
                        out=xt[:, cc * EXT + r0: cc * EXT + r0 + pn],
                        in_=tp[:, :pn],
                    )

            # ---- conv: 80 hidden matmuls accumulate into one PSUM bank ----
            psum_y = ps_y.tile([OC, Y_LOC], f32, tag="y")
            first = True
            for cc in range(16):
                for k in range(K):
                    nc.tensor.matmul(
                        out=psum_y[:],
                        lhsT=wtile[:, cc * 640 + k * 128: cc * 640 + (k + 1) * 128],
                        rhs=xt[:, cc * EXT + k: cc * EXT + k + Y_LOC],
                        start=first,
                        stop=False,
                    )
                    first = False

            # ---- pass 1: surprisal for EXT rows ----
            for r0, pn in ROW_TILES:
                ids_sb = sm.tile([128, 1], i32, tag="ids")
                nc.sync.dma_start(out=ids_sb[:pn, :], in_=ids[r0:r0 + pn, :])
                mask_sb = sm.tile([128, 1], f32, tag="mask")
                nc.sync.dma_start(out=mask_sb[:pn, :], in_=maskd[r0:r0 + pn, :])

                rowbase = sm.tile([128, 1], i32, tag="rowbase")
                nc.gpsimd.iota(rowbase[:pn, :], pattern=[[1, 1]],
                               base=r0 * V, channel_multiplier=V)
                flat_ids = sm.tile([128, 1], i32, tag="flatids")
                nc.vector.tensor_tensor(out=flat_ids[:pn, :], in0=ids_sb[:pn, :],
                                        in1=rowbase[:pn, :], op=Alu.add)
                gath = sm.tile([128, 1], f32, tag="gath")
                if VARIANT == "nogather":
                    nc.vector.memset(gath[:pn, :], 0.0)
                elif VARIANT == "flat2d":
                    nc.gpsimd.indirect_dma_start(
                        out=gath[:pn, :],
                        out_offset=None,
                        in_=logits,
                        in_offset=bass.IndirectOffsetOnAxis(
                            ap=flat_ids[:pn, :1], axis=1),
                    )
                else:
                    nc.gpsimd.indirect_dma_start(
                        out=gath[:pn, :],
                        out_offset=None,
                        in_=logits_flat,
                        in_offset=bass.IndirectOffsetOnAxis(
                            ap=flat_ids[:pn, :1], axis=0),
                    )

                sums = sm.tile([128, NCH], f32, tag="sums")
                for ci in range(NCH):
                    x_sb = lp.tile([128, CF], f32, tag="x")
                    nc.sync.dma_start(
                        out=x_sb[:pn, :],
                        in_=logits[r0:r0 + pn, ci * CF:(ci + 1) * CF],
                    )
                    e_sb = scr.tile([128, CF], f32, tag="e")
                    nc.scalar.activation(
                        out=e_sb[:pn, :], in_=x_sb[:pn, :], func=Act.Exp,
                        accum_out=sums[:pn, ci:ci + 1],
                    )

                sumexp = sm.tile([128, 1], f32, tag="sumexp")
                nc.vector.tensor_reduce(
                    out=sumexp[:pn, :], in_=sums[:pn, :],
                    axis=mybir.AxisListType.X, op=Alu.add,
                )
                lse = sm.tile([128, 1], f32, tag="lse")
                nc.scalar.activation(out=lse[:pn, :], in_=sumexp[:pn, :], func=Act.Ln)

                surp = sm.tile([128, 1], f32, tag="surp")
                nc.vector.tensor_tensor(out=surp[:pn, :], in0=lse[:pn, :],
                                        in1=gath[:pn, :], op=Alu.subtract)
                nc.vector.tensor_tensor(out=surp[:pn, :], in0=surp[:pn, :],
                                        in1=mask_sb[:pn, :], op=Alu.mult)
                nc.vector.tensor_scalar(out=surp[:pn, :], in0=surp[:pn, :],
                                        scalar1=LOG2E, scalar2=None, op0=Alu.mult)
                nc.sync.dma_start(out=surp_dram[0:1, r0:r0 + pn], in_=surp[:pn, :])

            # ---- surp channel: 5 rank-1-ish matmuls close the accumulation ----
            s5 = big.tile([K, Y_LOC], f32, tag="s5")
            for k in range(K):
                nc.sync.dma_start(out=s5[k:k + 1, :],
                                  in_=surp_dram[0:1, k:k + Y_LOC])
            nc.tensor.matmul(
                out=psum_y[:],
                lhsT=wsurp_sb[:],
                rhs=s5[:],
                start=False,
                stop=True,
            )

            # ---- maxpool(5) + bias + relu ----
            pooled = big.tile([OC, PO_LOC], f32, tag="pooled")
            stop_off = K * (PO_LOC - 1) + 1
            nc.vector.tensor_copy(out=pooled[:], in_=psum_y[:, 0:stop_off:K])
            for j in range(1, K):
                nc.vector.tensor_tensor(out=pooled[:], in0=pooled[:],
                                        in1=psum_y[:, j:j + stop_off:K], op=Alu.max)
            nc.vector.tensor_scalar(out=pooled[:], in0=pooled[:],
                                    scalar1=convb_sb[:, 0:1], scalar2=None,
                                    op0=Alu.add)
            nc.vector.tensor_scalar(out=pooled[:], in0=pooled[:],
                                    scalar1=0.0, scalar2=None, op0=Alu.max)

            # ---- FC partial: red[oc, l] = sum_p pooled*fcw ----
            red = big.tile([OC, 3], f32, tag="red")
            fc_scr = big.tile([OC, PO_LOC], f32, tag="fcscr")
            for l in range(3):
                nc.vector.tensor_tensor(
                    out=fc_scr[:],
                    in0=pooled[:],
                    in1=fcw_sb[:, l * PO_LOC:(l + 1) * PO_LOC],
                    op=Alu.mult,
                )
                nc.vector.tensor_reduce(
                    out=red[:, l:l + 1], in_=fc_scr[:],
                    axis=mybir.AxisListType.X, op=Alu.add,
                )
            # sentiment branch (zeroed on h==1 cores)
            rs = sm.tile([128, 1], f32, tag="rs")
            nc.vector.tensor_scalar(out=rs[:], in0=sentv_sb[:], scalar1=0.0,
                                    scalar2=None, op0=Alu.max)
            tmp3 = sm.tile([128, 3], f32, tag="tmp3")
            nc.vector.tensor_scalar(out=tmp3[:], in0=sentw_sb[:],
                                    scalar1=rs[:, 0:1], scalar2=None, op0=Alu.mult)
            nc.vector.tensor_tensor(out=red[:], in0=red[:], in1=tmp3[:], op=Alu.add)

            psum_out = ps_o.tile([3, 1], f32, tag="po")
            nc.tensor.matmul(out=psum_out[:], lhsT=red[:], rhs=ones_sb[:],
                             start=True, stop=True)
            out_sb = sm.tile([3, 1], f32, tag="outsb")
            nc.vector.tensor_tensor(out=out_sb[:], in0=psum_out[:], in1=fcb_sb[:],
                                    op=Alu.add)
            nc.sync.dma_start(out=out, in_=out_sb[:])

    nc.compile()
    return nc


def _prep_core_inputs(core, input_ids, attention_mask, sentiment, logits,
                      hidden, conv_w, conv_b, fc_w, fc_b):
    b, h = core // 2, core % 2
    g0 = Y_LOC * h
    ext0 = g0 - 2

    lg = np.zeros((EXT, V), np.float32)
    idl = np.zeros((EXT, 1), np.int32)
    mk = np.zeros((EXT, 1), np.float32)
    hd = np.zeros((EXT, H), np.float32)
    lo = max(0, -ext0)            # local index where valid rows start
    s0, s1 = ext0 + lo, ext0 + EXT
    lg[lo:] = logits[b, s0:s1]
    idl[lo:, 0] = input_ids[b, s0:s1].astype(np.int32)
    mk[lo:, 0] = attention_mask[b, s0:s1]
    hd[lo:] = hidden[b, s0:s1]

    wt = np.ascontiguousarray(
        conv_w[:, :H, :].transpose(1, 2, 0).reshape(H, K * OC))
    ws = np.ascontiguousarray(conv_w[:, H, :].T)           # [K, OC]
    cb = np.ascontiguousarray(conv_b[:, None])             # [OC, 1]

    w3 = fc_w[:, :OC * 204].reshape(3, OC, 204)
    fcw = np.ascontiguousarray(
        w3[:, :, h * PO_LOC:(h + 1) * PO_LOC].transpose(1, 0, 2).reshape(OC, 3 * PO_LOC))

    sv = np.zeros((128, 1), np.float32)
    sw = np.zeros((128, 3), np.float32)
    fb = np.zeros((3, 1), np.float32)
    if h == 0:
        sv[:3, 0] = sentiment[b]
        sw[:3, :] = fc_w[:, OC * 204:].T                   # [3 j, 3 l]
        fb[:, 0] = fc_b

    return {
        "logits_loc": lg, "ids_loc": idl, "mask_loc": mk, "hidden_loc": hd,
        "wt": wt, "wsurp": ws, "convb": cb, "fcw": fcw,
        "sentv": sv, "sentw": sw, "fcb": fb,
    }


def _install_ntff_hook():
    import sys
    import types
    try:
        import antenv
        from trn_agent_boot.trn_boot import _ntff_profile_via_ctypes
    except ImportError:
        return
    if "antenv.axon_hooks" in sys.modules:
        return
    mod = types.ModuleType("antenv.axon_hooks")
    _h = [None]
    mod.set_axon_ntff_profile_hook = lambda hk: _h.__setitem__(0, hk)
    mod.get_axon_ntff_profile_hook = lambda: _h[0]
    sys.modules["antenv.axon_hooks"] = mod
    antenv.axon_hooks = mod
    try:
        mod.set_axon_ntff_profile_hook(
            _ntff_profile_via_ctypes('/opt/axon/libaxon_pjrt.so'))
    except Exception:
        pass


def kernel(input_ids, attention_mask, sentiment, logits, hidden,
           conv_w, conv_b, fc_w, fc_b, _trace=False):
    from concourse.bass_utils import run_bass_kernel_spmd

    input_ids = np.asarray(input_ids)
    attention_mask = np.asarray(attention_mask, np.float32)
    sentiment = np.asarray(sentiment, np.float32)
    logits = np.asarray(logits, np.float32)
    hidden = np.asarray(hidden, np.float32)
    conv_w = np.asarray(conv_w, np.float32)
    conv_b = np.asarray(conv_b, np.float32)
    fc_w = np.asarray(fc_w, np.float32)
    fc_b = np.asarray(fc_b, np.float32)

    if "nc" not in _CACHE:
        _CACHE["nc"] = _build_program()
    nc = _CACHE["nc"]

    in_maps = [
        _prep_core_inputs(c, input_ids, attention_mask, sentiment, logits,
                          hidden, conv_w, conv_b, fc_w, fc_b)
        for c in range(N_CORES)
    ]
    if _trace:
        _install_ntff_hook()
    res = run_bass_kernel_spmd(nc, in_maps, list(range(N_CORES)), trace=_trace)
    _CACHE["last_result"] = res

    out = np.zeros((B, 3), np.float32)
    for b in range(B):
        out[b] = (res.results[2 * b]["out_loc"][:, 0]
                  + res.results[2 * b + 1]["out_loc"][:, 0])
    return out


# revision 18
# speedup vs baseline: 1.2330x; 1.0954x over previous
"""Trainium2 Bass kernel for nn_CNN_80221399155117.

Pipeline: full-vocab softmax -> token-prob gather -> -log2 surprisal ->
concat(hidden, surp) -> Conv1d(k=5, pad=2) -> MaxPool1d(5) -> ReLU -> FC.

Sharding: 8 cores = (batch b, seq-half h). Each core owns the pool-aligned
conv-output range [510h, 510h+510) of its batch, needing feats rows
[510h-2, 510h+512) (EXT=514, zero-padded outside [0,1024)). The softmax
normalizer is computed locally per row (positions sharded, vocab local),
so no collectives are needed. The token-logit gather runs on-device via
indirect DMA with flat indices built from iota + input_ids.
"""

import numpy as np

B, S, V, H = 4, 1024, 32000, 2048
OC, K = 128, 5
N_CORES = 8
Y_LOC = 510            # conv output positions per core (102 pool windows)
PO_LOC = 102           # pooled cols per core
EXT = 514              # feats rows incl conv halo (510 + 2 + 2)
CF = 4000              # vocab chunk (free-dim) size
NCH = V // CF          # 8 chunks
LOG2E = 1.4426950408889634

_CACHE = {}
VARIANT = "indirect"   # bisect knob: indirect | nogather | flat2d | nopass1 | noconv


def _build_program():
    import concourse.tile as tile
    from concourse import bacc, bass, mybir
    from concourse.masks import make_identity

    f32 = mybir.dt.float32
    i32 = mybir.dt.int32
    Alu = mybir.AluOpType
    Act = mybir.ActivationFunctionType

    nc = bacc.Bacc("TRN2", target_bir_lowering=False, debug=False,
                   num_devices=N_CORES)

    logits = nc.dram_tensor("logits_loc", [EXT, V], f32, kind="ExternalInput").ap()
    ids = nc.dram_tensor("ids_loc", [EXT, 1], i32, kind="ExternalInput").ap()
    maskd = nc.dram_tensor("mask_loc", [EXT, 1], f32, kind="ExternalInput").ap()
    hid = nc.dram_tensor("hidden_loc", [EXT, H], f32, kind="ExternalInput").ap()
    wt = nc.dram_tensor("wt", [H, K * OC], f32, kind="ExternalInput").ap()
    wsurp = nc.dram_tensor("wsurp", [K, OC], f32, kind="ExternalInput").ap()
    convb = nc.dram_tensor("convb", [OC, 1], f32, kind="ExternalInput").ap()
    fcw = nc.dram_tensor("fcw", [OC, 3 * PO_LOC], f32, kind="ExternalInput").ap()
    sentv = nc.dram_tensor("sentv", [128, 1], f32, kind="ExternalInput").ap()
    sentw = nc.dram_tensor("sentw", [128, 3], f32, kind="ExternalInput").ap()
    fcb = nc.dram_tensor("fcb", [3, 1], f32, kind="ExternalInput").ap()
    out = nc.dram_tensor("out_loc", [3, 1], f32, kind="ExternalOutput").ap()

    surp_dram = nc.dram_tensor("surp_scratch", [1, EXT], f32).ap()

    logits_flat = bass.AP(logits.tensor, 0, [[1, EXT * V], [1, 1]])

    ROW_TILES = [(0, 128), (128, 128), (256, 128), (384, 128)]
    NHALO = EXT - 512                  # 2 halo rows, packed [128, HF]
    HQ = 128 // NHALO                  # partitions per halo row
    HF = V // HQ                       # free elems per partition

    with tile.TileContext(nc) as tc:
        with (
            tc.tile_pool(name="lp", bufs=6) as lp,          # logits chunks
            tc.tile_pool(name="scr", bufs=2) as scr,        # exp scratch
            tc.tile_pool(name="big", bufs=1) as big,        # resident X / weights
            tc.tile_pool(name="hn", bufs=2) as hnp,         # hidden natural tiles
            tc.tile_pool(name="sm", bufs=12) as sm,         # small per-tile stats
            tc.tile_pool(name="ps_t", bufs=4, space="PSUM") as ps_t,
            tc.tile_pool(name="ps_y", bufs=1, space="PSUM") as ps_y,
            tc.tile_pool(name="ps_o", bufs=1, space="PSUM") as ps_o,
        ):
            # ---- resident constants ----
            ident = big.tile([128, 128], f32, tag="ident")
            make_identity(nc, ident[:])
            f16 = mybir.dt.float16
            wtile = big.tile([128, 16 * K * OC], f16, tag="wtile")  # 16 ch-chunks
            for cc in range(16):
                nc.gpsimd.dma_start(        # SWDGE casts f32->bf16 in flight
                    out=wtile[:, cc * 640:(cc + 1) * 640],
                    in_=wt[cc * 128:(cc + 1) * 128, :],
                )
            wsurp_sb = big.tile([K, OC], f32, tag="wsurp")
            nc.sync.dma_start(out=wsurp_sb[:], in_=wsurp)
            convb_sb = big.tile([OC, 1], f32, tag="convb")
            nc.sync.dma_start(out=convb_sb[:], in_=convb)
            fcw_sb = big.tile([OC, 3 * PO_LOC], f32, tag="fcw")
            nc.sync.dma_start(out=fcw_sb[:], in_=fcw)
            sentv_sb = big.tile([128, 1], f32, tag="sentv")
            nc.sync.dma_start(out=sentv_sb[:], in_=sentv)
            sentw_sb = big.tile([128, 3], f32, tag="sentw")
            nc.sync.dma_start(out=sentw_sb[:], in_=sentw)
            fcb_sb = big.tile([3, 1], f32, tag="fcb")
            nc.sync.dma_start(out=fcb_sb[:], in_=fcb)
            ones_sb = big.tile([128, 1], f32, tag="ones")
            nc.vector.memset(ones_sb[:], 1.0)

            # ---- hidden -> transposed X tiles [ch, pos] ----
            xt = big.tile([128, 16 * EXT], f16, tag="xt")
            for r0, pn in ROW_TILES + [(512, NHALO)]:
                hn = hnp.tile([128, H], f32, tag="hn")
                nc.sync.dma_start(out=hn[:pn, :], in_=hid[r0:r0 + pn, :])
                for cc in range(16):
                    tp = ps_t.tile([128, 128], f32, tag="tp")
                    nc.tensor.transpose(
                        out=tp[:, :pn],
                        in_=hn[:pn, cc * 128:(cc + 1) * 128],
                        identity=ident[:pn, :pn],
                    )
                    nc.vector.tensor_copy(
                        out=xt[:, cc * EXT + r0: cc * EXT + r0 + pn],
                        in_=tp[:, :pn],
                    )

            # ---- conv: 80 hidden matmuls accumulate into one PSUM bank ----
            psum_y = ps_y.tile([OC, Y_LOC], f32, tag="y")
            first = True
            for cc in range(16):
                for k in range(K):
                    nc.tensor.matmul(
                        out=psum_y[:],
                        lhsT=wtile[:, cc * 640 + k * 128: cc * 640 + (k + 1) * 128],
                        rhs=xt[:, cc * EXT + k: cc * EXT + k + Y_LOC],
                        start=first,
                        stop=False,
                    )
                    first = False

            # ---- halo rows (2): vocab packed across partitions ----
            # layout [128, HF]: partition p = (row a=p//HQ, slice q=p%HQ)
            hx = lp.tile([128, HF], f32, tag="x")
            halo_src = bass.AP(logits.tensor, 512 * V,
                               [[V, NHALO], [HF, HQ], [1, HF]])
            nc.sync.dma_start(out=hx[:], in_=halo_src)
            hscr = scr.tile([128, HF], f32, tag="e")
            hsums = sm.tile([128, 1], f32, tag="hsums")
            nc.scalar.activation(out=hscr[:], in_=hx[:], func=Act.Exp,
                                 accum_out=hsums[:])
            hsel = big.tile([128, NHALO], f32, tag="hsel")
            nc.vector.memset(hsel[:], 0.0)
            for a in range(NHALO):
                nc.vector.memset(hsel[a * HQ:(a + 1) * HQ, a:a + 1], 1.0)
            psum_h = ps_o.tile([NHALO, 1], f32, tag="ph")
            nc.tensor.matmul(out=psum_h[:], lhsT=hsel[:], rhs=hsums[:],
                             start=True, stop=True)
            # ids/mask/gather/lse/surp for the 2 halo rows
            hids = sm.tile([128, 1], i32, tag="ids")
            nc.sync.dma_start(out=hids[:NHALO, :], in_=ids[512:EXT, :])
            hmask = sm.tile([128, 1], f32, tag="mask")
            nc.sync.dma_start(out=hmask[:NHALO, :], in_=maskd[512:EXT, :])
            hrb = sm.tile([128, 1], i32, tag="rowbase")
            nc.gpsimd.iota(hrb[:NHALO, :], pattern=[[1, 1]], base=512 * V,
                           channel_multiplier=V)
            hfl = sm.tile([128, 1], i32, tag="flatids")
            nc.vector.tensor_tensor(out=hfl[:NHALO, :], in0=hids[:NHALO, :],
                                    in1=hrb[:NHALO, :], op=Alu.add)
            hg = sm.tile([128, 1], f32, tag="gath")
            nc.gpsimd.indirect_dma_start(
                out=hg[:NHALO, :], out_offset=None, in_=logits_flat,
                in_offset=bass.IndirectOffsetOnAxis(ap=hfl[:NHALO, :1], axis=0))
            hlse = sm.tile([128, 1], f32, tag="lse")
            nc.scalar.activation(out=hlse[:NHALO, :], in_=psum_h[:], func=Act.Ln)
            hsurp = sm.tile([128, 1], f32, tag="surp")
            nc.vector.tensor_tensor(out=hsurp[:NHALO, :], in0=hlse[:NHALO, :],
                                    in1=hg[:NHALO, :], op=Alu.subtract)
            nc.vector.tensor_tensor(out=hsurp[:NHALO, :], in0=hsurp[:NHALO, :],
                                    in1=hmask[:NHALO, :], op=Alu.mult)
            nc.vector.tensor_scalar(out=hsurp[:NHALO, :], in0=hsurp[:NHALO, :],
                                    scalar1=LOG2E, scalar2=None, op0=Alu.mult)
            nc.sync.dma_start(out=surp_dram[0:1, 512:EXT], in_=hsurp[:NHALO, :])

            # ---- pass 1: surprisal for main 512 rows ----
            for r0, pn in ROW_TILES:
                ids_sb = sm.tile([128, 1], i32, tag="ids")
                nc.sync.dma_start(out=ids_sb[:pn, :], in_=ids[r0:r0 + pn, :])
                mask_sb = sm.tile([128, 1], f32, tag="mask")
                nc.sync.dma_start(out=mask_sb[:pn, :], in_=maskd[r0:r0 + pn, :])

                rowbase = sm.tile([128, 1], i32, tag="rowbase")
                nc.gpsimd.iota(rowbase[:pn, :], pattern=[[1, 1]],
                               base=r0 * V, channel_multiplier=V)
                flat_ids = sm.tile([128, 1], i32, tag="flatids")
                nc.vector.tensor_tensor(out=flat_ids[:pn, :], in0=ids_sb[:pn, :],
                                        in1=rowbase[:pn, :], op=Alu.add)
                gath = sm.tile([128, 1], f32, tag="gath")
                if VARIANT == "nogather":
                    nc.vector.memset(gath[:pn, :], 0.0)
                elif VARIANT == "flat2d":
                    nc.gpsimd.indirect_dma_start(
                        out=gath[:pn, :],
                        out_offset=None,
                        in_=logits,
                        in_offset=bass.IndirectOffsetOnAxis(
                            ap=flat_ids[:pn, :1], axis=1),
                    )
                else:
                    nc.gpsimd.indirect_dma_start(
                        out=gath[:pn, :],
                        out_offset=None,
                        in_=logits_flat,
                        in_offset=bass.IndirectOffsetOnAxis(
                            ap=flat_ids[:pn, :1], axis=0),
                    )

                sums = sm.tile([128, NCH], f32, tag="sums")
                for ci in range(NCH):
                    x_sb = lp.tile([128, CF], f32, tag="x")
                    nc.sync.dma_start(
                        out=x_sb[:pn, :],
                        in_=logits[r0:r0 + pn, ci * CF:(ci + 1) * CF],
                    )
                    e_sb = scr.tile([128, CF], f32, tag="e")
                    nc.scalar.activation(
                        out=e_sb[:pn, :], in_=x_sb[:pn, :], func=Act.Exp,
                        accum_out=sums[:pn, ci:ci + 1],
                    )

                sumexp = sm.tile([128, 1], f32, tag="sumexp")
                nc.vector.tensor_reduce(
                    out=sumexp[:pn, :], in_=sums[:pn, :],
                    axis=mybir.AxisListType.X, op=Alu.add,
                )
                lse = sm.tile([128, 1], f32, tag="lse")
                nc.scalar.activation(out=lse[:pn, :], in_=sumexp[:pn, :], func=Act.Ln)

                surp = sm.tile([128, 1], f32, tag="surp")
                nc.vector.tensor_tensor(out=surp[:pn, :], in0=lse[:pn, :],
                                        in1=gath[:pn, :], op=Alu.subtract)
                nc.vector.tensor_tensor(out=surp[:pn, :], in0=surp[:pn, :],
                                        in1=mask_sb[:pn, :], op=Alu.mult)
                nc.vector.tensor_scalar(out=surp[:pn, :], in0=surp[:pn, :],
                                        scalar1=LOG2E, scalar2=None, op0=Alu.mult)
                nc.sync.dma_start(out=surp_dram[0:1, r0:r0 + pn], in_=surp[:pn, :])

            # ---- surp channel: 5 rank-1-ish matmuls close the accumulation ----
            s5 = big.tile([K, Y_LOC], f32, tag="s5")
            for k in range(K):
                nc.sync.dma_start(out=s5[k:k + 1, :],
                                  in_=surp_dram[0:1, k:k + Y_LOC])
            nc.tensor.matmul(
                out=psum_y[:],
                lhsT=wsurp_sb[:],
                rhs=s5[:],
                start=False,
                stop=True,
            )

            # ---- maxpool(5) + bias + relu ----
            pooled = big.tile([OC, PO_LOC], f32, tag="pooled")
            stop_off = K * (PO_LOC - 1) + 1
            nc.vector.tensor_copy(out=pooled[:], in_=psum_y[:, 0:stop_off:K])
            for j in range(1, K):
                nc.vector.tensor_tensor(out=pooled[:], in0=pooled[:],
                                        in1=psum_y[:, j:j + stop_off:K], op=Alu.max)
            nc.vector.tensor_scalar(out=pooled[:], in0=pooled[:],
                                    scalar1=convb_sb[:, 0:1], scalar2=None,
                                    op0=Alu.add)
            nc.vector.tensor_scalar(out=pooled[:], in0=pooled[:],
                                    scalar1=0.0, scalar2=None, op0=Alu.max)

            # ---- FC partial: red[oc, l] = sum_p pooled*fcw ----
            red = big.tile([OC, 3], f32, tag="red")
            fc_scr = big.tile([OC, PO_LOC], f32, tag="fcscr")
            for l in range(3):
                nc.vector.tensor_tensor(
                    out=fc_scr[:],
                    in0=pooled[:],
                    in1=fcw_sb[:, l * PO_LOC:(l + 1) * PO_LOC],
                    op=Alu.mult,
                )
                nc.vector.tensor_reduce(
                    out=red[:, l:l + 1], in_=fc_scr[:],
                    axis=mybir.AxisListType.X, op=Alu.add,
                )
            # sentiment branch (zeroed on h==1 cores)
            rs = sm.tile([128, 1], f32, tag="rs")
            nc.vector.tensor_scalar(out=rs[:], in0=sentv_sb[:], scalar1=0.0,
                                    scalar2=None, op0=Alu.max)
            tmp3 = sm.tile([128, 3], f32, tag="tmp3")
            nc.vector.tensor_scalar(out=tmp3[:], in0=sentw_sb[:],
                                    scalar1=rs[:, 0:1], scalar2=None, op0=Alu.mult)
            nc.vector.tensor_tensor(out=red[:], in0=red[:], in1=tmp3[:], op=Alu.add)

            psum_out = ps_o.tile([3, 1], f32, tag="po")
            nc.tensor.matmul(out=psum_out[:], lhsT=red[:], rhs=ones_sb[:],
                             start=True, stop=True)
            out_sb = sm.tile([3, 1], f32, tag="outsb")
            nc.vector.tensor_tensor(out=out_sb[:], in0=psum_out[:], in1=fcb_sb[:],
                                    op=Alu.add)
            nc.sync.dma_start(out=out, in_=out_sb[:])

    nc.compile()
    return nc


def _prep_core_inputs(core, input_ids, attention_mask, sentiment, logits,
                      hidden, conv_w, conv_b, fc_w, fc_b):
    b, h = core // 2, core % 2
    g0 = Y_LOC * h
    ext0 = g0 - 2

    lg = np.zeros((EXT, V), np.float32)
    idl = np.zeros((EXT, 1), np.int32)
    mk = np.zeros((EXT, 1), np.float32)
    hd = np.zeros((EXT, H), np.float32)
    lo = max(0, -ext0)            # local index where valid rows start
    s0, s1 = ext0 + lo, ext0 + EXT
    lg[lo:] = logits[b, s0:s1]
    idl[lo:, 0] = input_ids[b, s0:s1].astype(np.int32)
    mk[lo:, 0] = attention_mask[b, s0:s1]
    hd[lo:] = hidden[b, s0:s1]

    wt = np.ascontiguousarray(
        conv_w[:, :H, :].transpose(1, 2, 0).reshape(H, K * OC))
    ws = np.ascontiguousarray(conv_w[:, H, :].T)           # [K, OC]
    cb = np.ascontiguousarray(conv_b[:, None])             # [OC, 1]

    w3 = fc_w[:, :OC * 204].reshape(3, OC, 204)
    fcw = np.ascontiguousarray(
        w3[:, :, h * PO_LOC:(h + 1) * PO_LOC].transpose(1, 0, 2).reshape(OC, 3 * PO_LOC))

    sv = np.zeros((128, 1), np.float32)
    sw = np.zeros((128, 3), np.float32)
    fb = np.zeros((3, 1), np.float32)
    if h == 0:
        sv[:3, 0] = sentiment[b]
        sw[:3, :] = fc_w[:, OC * 204:].T                   # [3 j, 3 l]
        fb[:, 0] = fc_b

    return {
        "logits_loc": lg, "ids_loc": idl, "mask_loc": mk, "hidden_loc": hd,
        "wt": wt, "wsurp": ws, "convb": cb, "fcw": fcw,
        "sentv": sv, "sentw": sw, "fcb": fb,
    }


def _install_ntff_hook():
    import sys
    import types
    try:
        import antenv
        from trn_agent_boot.trn_boot import _ntff_profile_via_ctypes
    except ImportError:
        return
    if "antenv.axon_hooks" in sys.modules:
        return
    mod = types.ModuleType("antenv.axon_hooks")
    _h = [None]
    mod.set_axon_ntff_profile_hook = lambda hk: _h.__setitem__(0, hk)
    mod.get_axon_ntff_profile_hook = lambda: _h[0]
    sys.modules["antenv.axon_hooks"] = mod
    antenv.axon_hooks = mod
    try:
        mod.set_axon_ntff_profile_hook(
            _ntff_profile_via_ctypes('/opt/axon/libaxon_pjrt.so'))
    except Exception:
        pass


def kernel(input_ids, attention_mask, sentiment, logits, hidden,
           conv_w, conv_b, fc_w, fc_b, _trace=False):
    from concourse.bass_utils import run_bass_kernel_spmd

    input_ids = np.asarray(input_ids)
    attention_mask = np.asarray(attention_mask, np.float32)
    sentiment = np.asarray(sentiment, np.float32)
    logits = np.asarray(logits, np.float32)
    hidden = np.asarray(hidden, np.float32)
    conv_w = np.asarray(conv_w, np.float32)
    conv_b = np.asarray(conv_b, np.float32)
    fc_w = np.asarray(fc_w, np.float32)
    fc_b = np.asarray(fc_b, np.float32)

    if "nc" not in _CACHE:
        _CACHE["nc"] = _build_program()
    nc = _CACHE["nc"]

    in_maps = [
        _prep_core_inputs(c, input_ids, attention_mask, sentiment, logits,
                          hidden, conv_w, conv_b, fc_w, fc_b)
        for c in range(N_CORES)
    ]
    if _trace:
        _install_ntff_hook()
    res = run_bass_kernel_spmd(nc, in_maps, list(range(N_CORES)), trace=_trace)
    _CACHE["last_result"] = res

    out = np.zeros((B, 3), np.float32)
    for b in range(B):
        out[b] = (res.results[2 * b]["out_loc"][:, 0]
                  + res.results[2 * b + 1]["out_loc"][:, 0])
    return out
